# revision 3
# baseline (speedup 1.0000x reference)
"""DQS encoder (dual-quaternion skinning blend) Trainium2 kernel, v3.

Contract: kernel(x, weights, VR) -> (8_000_000,) float32, matching
reference._dqs numerics. Data-parallel over nodes across 8 NeuronCores.

The 2e-2 rel-err budget admits an fp16 pipeline end to end (measured
2.1e-3 vs the fp32 reference; bf16 would be 2.1e-2 -- the blend's
catastrophic cancellation for small |q| needs fp16's 11-bit mantissa).
fp16 halves weight/VR/output HBM traffic, enables the xbar transpose
DMA (2-byte dtypes only), and doubles DVE throughput (2x_2p mode needs
2-byte dtypes, all-SBUF operands, packed innermost dims).

Host side:
  - weights are cast to fp16 and repacked per 12288-node window into
    (2048, 128) xbar-transpose source blocks: row c = 512j + 128u + p
    holds the 20-weight rows of nodes {288p + 96wb + 24u + 6j + nw,
    nw=0..5} of the window's 36864-node group (120 fp16 + 8 zero pad).
  - x -> block-diag blend stationary bd[20nw+k, 6cc+nw] = qm4[k, cc],
    fp16 (128, 32), rows 120..127 zero so the pad lanes contribute 0.
  - VR is de-interleaved to fp16 component planes vrp[4, npc] so the
    on-chip component views have packed innermost dims (DVE 2x).
  - output comes back as fp16 planes; host re-interleaves + upcasts.

Per-core pipeline (250k nodes; group = 3 windows = 36864 consecutive
nodes, partition p owns nodes 288p..288p+287 of its group; the last
group overlaps its predecessor -- idempotent rewrites):
  1. Per window, one HWDGE xbar DMA-transpose (2048,128)f16 ->
     strip[128, 2048]: strip[20nw+k, c] = weight k of node nw, group c.
  2. 4 single fp16 matmuls (K=128, M=32 at base 0/32 of two PSUM banks,
     N=512): qs[32j + 6cc + nw, n] = qs_cc(node(512j+n, nw)).
  3. Drain both banks to qssb[128, 512] fp32; 4 PE transposes of its
     128-col chunks -> tps; ACT-drain to fp16
     T[p, 512wb + 128u + 32j + 6cc + nw] = qs_cc(288p+96wb+24u+6j+nw).
  4. One VR load per group: vrt[p, 288c + m] = VR_c(group + 288p + m)
     (4 x 576 B contiguous runs per partition -- every DMA descriptor
     in the kernel is >=512 B, dodging the sub-512B HBM penalty).
  5. fp16 DVE math over the group (cross-product rotation form):
       t = u x v + d*v ;  y = v + (2/|q|^2) * (u x t)
     with |q|^2/2 via ACT Square(scale=sqrt(.5)) + DVE reduce, 2/|q|^2
     via DVE reciprocal; y written in place over the v planes.
  6. One store per group (same planar AP mirrored).

Walrus's codegen accepts only ONE sync-wait command on several
instruction encodings. Countermeasures: SWDGE completion semaphores
collapsed to one lane; same-sem waits collapsed to their max value; and
traced nop placeholders seeded per engine -- a post-schedule pass moves
any excess waits onto a placeholder relocated just before the
instruction (same-engine program order then enforces the dependency).
"""

import sys
from contextlib import ExitStack

import numpy as np

sys.path.insert(0, "/opt/trn_rl_repo")

import concourse.bass as bass  # noqa: E402
import concourse.tile as tile  # noqa: E402
from concourse import mybir  # noqa: E402
from concourse import tile_sem_assignment as _tsa  # noqa: E402
from concourse.bass_utils import run_bass_kernel_spmd  # noqa: E402

# One SWDGE completion lane: DMA ticks subsume each other (the SWDGE queue
# is FIFO), so no instruction ever needs two DMA waits.
_tsa.NUM_SWDGE_GLOBAL_SEMS = 1

FP = mybir.dt.float32
HF = mybir.dt.float16
OP = mybir.AluOpType

N_NODES = 2_000_000
N_CORES = 8
NPC = N_NODES // N_CORES  # 250_000 nodes per core

G_NODES = 6          # nodes per strip column
KW = 20              # weights per node
NW = 512             # matmul moving free dim
WIN = 12288          # nodes per window (2048 groups of 6)
GPW = WIN // G_NODES  # 2048 strip columns per window
B_DEF = 3            # windows per math group
GRP = B_DEF * WIN    # 36864 nodes per group
NPP = GRP // 128     # 288 nodes per partition per group


def _group_bases(npc):
    n_full = npc // GRP
    bases = [g * GRP for g in range(n_full)]
    if npc - n_full * GRP > 0:
        bases.append(npc - GRP)  # overlapped tail group (idempotent writes)
    return bases


def _fview(ap, off, dims):
    """Strided free-dim view of a 2-D SBUF/PSUM AP. dims = [[step, count],...]."""
    return bass.AP(tensor=ap.tensor, offset=ap.offset + off, ap=[ap.ap[0]] + dims)


class _Ph:
    """Pool of traced carrier nops for the excess-wait retarget pass.
    A nop with no semaphore effects is position-neutral, so the post-pass
    relocates them to just before any instruction that needs a wait
    peeled off."""

    def __init__(self):
        self.names = set()

    def pre_alloc(self, nc, n):
        # Allocate the junk buffer BEFORE the TileContext opens so the
        # pool allocator routes around it (a post-context
        # alloc_sbuf_tensor lands inside released pool space and the
        # carrier memsets then clobber live tiles -- found by CoreSim's
        # race detector), and so the memsets' APs are physical (pool-tile
        # APs are symbolic and don't serialize post-context).
        self._junk = nc.alloc_sbuf_tensor(
            "ph_junk_dve", [1, n], mybir.dt.float32)
        self._n = n

    def seed(self, nc, n_per_engine=96):
        sem = nc.alloc_semaphore("ph_carrier_dummy")
        for ns in (nc.tensor, nc.gpsimd, nc.scalar, nc.sync):
            for _ in range(n_per_engine):
                p = ns.wait_ge(sem, 0).ins
                self.names.add(p.name)

    def seed_dve_late(self, nc):
        """DVE EventSemaphores don't survive walrus codegen, and memsets
        emitted inside the TileContext acquire scheduler sem ticks (which
        pin their position). Emit junk memsets AFTER the context closes:
        they stay sync-free, hence position-neutral."""
        for k in range(self._n):
            p = nc.vector.memset(self._junk[0:1, k : k + 1], 0.0).ins
            self.names.add(p.name)


def _retarget_waits(nc, ph_names):
    """Walrus accepts only one sync-wait command on several instruction
    encodings. Collapse same-sem waits to their max value; for every
    instruction still holding N>1 waits, relocate N-1 seeded carrier
    nops (semaphore-free, hence position-neutral) to just before it and
    move the excess waits onto them; same-engine program order then
    enforces the dependency."""
    import bass_rust

    moved = 0
    skip = ("InstEventSemaphore", "InstNoOp")
    allow = ("InstMatmult", "InstActivation", "InstDMACopy", "InstDrain",
             "InstTensorTensor", "InstTensorScalarPtr", "InstTensorReduce",
             "InstReciprocal", "InstCopy", "InstTensorCopy",
             "InstDmaTransposeAnt", "InstMemset")
    blocks = list(nc.main_func.blocks)
    pool = {}
    plan = {}
    consumed = set()
    for bb in blocks:
        for ins in bb.instructions:
            if ins.name in ph_names and (
                ins.sync_info is None or not ins.sync_info.on_update
            ):
                pool.setdefault(ins.engine, []).append(ins)
    for bb in blocks:
        for ins in bb.instructions:
            if ins.name in ph_names:
                continue
            if type(ins).__name__ in skip or type(ins).__name__ not in allow:
                continue
            si = ins.sync_info
            if si is not None and len(si.on_wait) > 1:
                # Same-sem waits are subsumed by the max value (monotone
                # sems) -- collapse before spending carriers.
                bysem = {}
                for w in si.on_wait:
                    k = w.ant_name
                    if k not in bysem or w.wait_value > bysem[k].wait_value:
                        bysem[k] = w
                waits = list(bysem.values())
                if len(waits) == 1:
                    ins.sync_info = bass_rust.SyncInfo(
                        on_wait=waits, on_update=list(si.on_update)
                    )
                    continue
                excess = waits[:-1]
                phs = pool.get(ins.engine, [])
                if len(phs) < len(excess):
                    raise RuntimeError(
                        f"{ins.name} ({type(ins).__name__} on {ins.engine}) "
                        f"needs {len(excess)} carriers, have {len(phs)}; "
                        f"waits={[(w.ant_name, w.wait_value) for w in waits]}"
                    )
                carriers = []
                for w in excess:
                    p = phs.pop()
                    p.sync_info = bass_rust.SyncInfo(on_wait=[w], on_update=[])
                    try:
                        p.bass_scheduled_tick = ins.bass_scheduled_tick
                    except Exception:
                        pass
                    consumed.add(p.name)
                    carriers.append(p)
                    moved += 1
                ins.sync_info = bass_rust.SyncInfo(
                    on_wait=waits[-1:], on_update=list(si.on_update)
                )
                plan[ins.name] = carriers
    unused = set()
    for phs in pool.values():
        unused.update(p.name for p in phs)
    for bb in blocks:
        out = []
        for ins in bb.instructions:
            if ins.name in consumed or ins.name in unused:
                continue
            out.extend(plan.get(ins.name, ()))
            out.append(ins)
        bb.instructions = out
    return moved


def build_program(npc=NPC, repeats=1, split_waits=True):
    nc = bass.Bass()

    gbases = _group_bases(npc)
    n_grp = len(gbases)
    n_win = n_grp * B_DEF

    # (repeats-1) junk pad rows make the program's input signature unique
    # per repeat count -- otherwise the R=1 and R=9 programs lower to
    # identical HLO and the PJRT compile cache silently serves the same
    # NEFF for both, nulling the repeat-slope timing method.
    wt_d = nc.dram_tensor("wt", [n_win * GPW + (repeats - 1), 128], HF,
                          kind="ExternalInput")
    vrp_d = nc.dram_tensor("vrp", [4 * npc], HF, kind="ExternalInput")
    bd_d = nc.dram_tensor("bd", [128, 32], HF, kind="ExternalInput")
    idf_d = nc.dram_tensor("idf", [128, 128], FP, kind="ExternalInput")
    outp_d = nc.dram_tensor("outp", [4 * npc], HF, kind="ExternalOutput")

    groups = list(enumerate(gbases)) * repeats

    ph = _Ph()
    ph.pre_alloc(nc, 96 + 96 * repeats)

    def planar_ap(dram, gb):
        """(p, c, m) access pattern over a [4*npc] fp16 plane tensor:
        element (p, c, m) = plane c of node gb + 288p + m."""
        full = dram[0 : 4 * npc]
        return bass.AP(
            tensor=full.tensor, offset=full.offset + gb,
            ap=[[NPP, 128], [npc, 4], [1, NPP]],
        )

    with tile.TileContext(nc) as tc, ExitStack() as ctx:
        const = ctx.enter_context(tc.tile_pool(name="const", bufs=1))
        ph.seed(nc, n_per_engine=96 + 96 * repeats)
        strip_p = ctx.enter_context(tc.tile_pool(name="strip", bufs=6))
        qssb_p = ctx.enter_context(tc.tile_pool(name="qssb", bufs=3))
        grp_p = ctx.enter_context(tc.tile_pool(name="grp", bufs=4))
        scr_p = ctx.enter_context(tc.tile_pool(name="scr", bufs=2))
        qpsa_p = ctx.enter_context(tc.tile_pool(name="qpsa", bufs=3, space="PSUM"))
        qpsb_p = ctx.enter_context(tc.tile_pool(name="qpsb", bufs=3, space="PSUM"))
        tps_p = ctx.enter_context(tc.tile_pool(name="tps", bufs=2, space="PSUM"))

        ident_f = const.tile([128, 128], FP)
        nc.gpsimd.dma_start(out=ident_f[:], in_=idf_d[:, :])
        bd_sb = const.tile([128, 32], HF)
        nc.gpsimd.dma_start(out=bd_sb[:], in_=bd_d[:, :])

        def emit_loads(gi_gb):
            gi, gb = gi_gb
            strips = []
            for wb in range(B_DEF):
                wi = (gi % n_grp) * B_DEF + wb
                strip = strip_p.tile([128, GPW], HF, tag="strip")
                nc.sync.dma_start_transpose(
                    strip[:], wt_d[wi * GPW : (wi + 1) * GPW, :]
                )
                strips.append(strip)
            vrt = grp_p.tile([128, 4 * NPP], HF, tag="vrt")
            vdst = vrt[:].rearrange("p (c m) -> p c m", c=4)
            nc.scalar.dma_start(vdst, planar_ap(vrp_d, gb))
            return strips, vrt

        def emit_blend(strips):
            t_sb = grp_p.tile([128, B_DEF * 512], HF, tag="t_sb")
            for wb in range(B_DEF):
                strip = strips[wb]
                qsa = qpsa_p.tile([64, NW], FP, tag="qsa")
                qsb = qpsb_p.tile([64, NW], FP, tag="qsb")
                for j, (tl, off) in enumerate(
                    ((qsa, 0), (qsa, 32), (qsb, 0), (qsb, 32))
                ):
                    nc.tensor.matmul(
                        tl[off : off + 32, :], bd_sb[:],
                        strip[:, NW * j : NW * (j + 1)],
                        start=True, stop=True,
                    )
                qssb = qssb_p.tile([128, NW], FP, tag="qssb")
                nc.scalar.copy(qssb[0:64, :], qsa[:])
                nc.scalar.copy(qssb[64:128, :], qsb[:])

                tps = tps_p.tile([128, 512], FP, tag="tps")
                for u in range(4):
                    nc.tensor.transpose(
                        tps[:, 128 * u : 128 * (u + 1)],
                        qssb[:, 128 * u : 128 * (u + 1)],
                        ident_f[:],
                    )
                # fp32 PSUM -> fp16 T (cast on drain)
                nc.scalar.copy(t_sb[:, 512 * wb : 512 * (wb + 1)], tps[:])
            return t_sb

        # software pipeline: group g+1's DMAs are issued before group g's
        # blend/math so every engine's in-order queue sees its work with
        # dependencies already satisfied.
        pending = emit_loads(groups[0]) if groups else None
        for gi, (_, gb) in enumerate(groups):
            strips, vrt = pending
            nxt = emit_loads(groups[gi + 1]) if gi + 1 < len(groups) else None
            t_sb = emit_blend(strips)
            pending = nxt

            # ---------- fp16 math over the group ----------
            a12 = 4 * B_DEF  # merged (window, u) dim
            fd = 96 * B_DEF

            def qv(cc):  # quat component plane view of t_sb
                return _fview(t_sb[:], 6 * cc, [[128, a12], [32, 4], [1, 6]])

            def vv(c):  # VR component plane view
                return _fview(vrt[:], NPP * c, [[24, a12], [6, 4], [1, 6]])

            def sh(tl):  # scratch tile shaped to match views
                return tl[:, :fd].rearrange("p (a j g) -> p a j g", a=a12, j=4)

            A, Bq, C, D = qv(0), qv(1), qv(2), qv(3)
            v1, v2, v3 = vv(0), vv(1), vv(2)

            t1 = scr_p.tile([128, fd], HF, tag="t1")
            t2 = scr_p.tile([128, fd], HF, tag="t2")
            t3 = scr_p.tile([128, fd], HF, tag="t3")
            s1 = scr_p.tile([128, fd], HF, tag="s1")
            s2 = scr_p.tile([128, fd], HF, tag="s2")
            w1 = scr_p.tile([128, fd], HF, tag="w1")
            w2 = scr_p.tile([128, fd], HF, tag="w2")
            w3 = scr_p.tile([128, fd], HF, tag="w3")
            n2 = scr_p.tile([128, fd], HF, tag="n2")
            inv = scr_p.tile([128, fd], HF, tag="inv")
            sq = scr_p.tile([128, 4 * fd], HF, tag="sq")

            def tt(out, a, b, op):
                nc.vector.tensor_tensor(out, a, b, op)

            # sq = (T/sqrt(2))^2 so n2 = |q|^2/2 and inv = 2/|q|^2,
            # folding the rotation's factor 2 into the reciprocal.
            sq_in = _fview(t_sb[:], 0, [[128, a12], [32, 4], [1, 24]])
            sq_out = sq[:, : 4 * fd].rearrange(
                "p (a j q) -> p a j q", a=a12, j=4
            )
            nc.scalar.activation(
                sq_out, sq_in, mybir.ActivationFunctionType.Square,
                scale=float(np.sqrt(0.5)),
            )
            sqr = _fview(sq[:], 0, [[24, 4 * a12], [1, 6], [6, 4]])
            # n2 in fp16 is safe: |q|^2/2 in [~5e-4, ~60], rel err 5e-4,
            # validated against the fp32 reference at 2.1e-3 end to end.
            with nc.allow_low_precision(reason="fp16 |q|^2 validated"):
                nc.vector.tensor_reduce(
                    out=sh(n2), in_=sqr, axis=mybir.AxisListType.X, op=OP.add
                )
            with nc.allow_low_precision(reason="fp16 2/|q|^2 validated"):
                nc.vector.reciprocal(out=inv[:, :fd], in_=n2[:, :fd])

            # t = u x v + d*v  (t1 = B*v3 - C*v2 + D*v1, etc.)
            for tout, (f1, e1), (f2, e2), (f3, e3) in (
                (t1, (Bq, v3), (C, v2), (D, v1)),
                (t2, (C, v1), (A, v3), (D, v2)),
                (t3, (A, v2), (Bq, v1), (D, v3)),
            ):
                tt(sh(s1), f1, e1, OP.mult)
                tt(sh(s2), f2, e2, OP.mult)
                tt(sh(s1), sh(s1), sh(s2), OP.subtract)
                tt(sh(s2), f3, e3, OP.mult)
                tt(sh(tout), sh(s1), sh(s2), OP.add)

            # w = u x t (the rotation's factor 2 lives in inv)
            for wout, (f1, e1), (f2, e2) in (
                (w1, (Bq, t3), (C, t2)),
                (w2, (C, t1), (A, t3)),
                (w3, (A, t2), (Bq, t1)),
            ):
                tt(sh(s1), f1, sh(e1), OP.mult)
                tt(sh(s2), f2, sh(e2), OP.mult)
                tt(sh(wout), sh(s1), sh(s2), OP.subtract)

            # y_c = v_c + inv * w_c   (written in place over v_c)
            for wsrc, vdst_ in ((w1, v1), (w2, v2), (w3, v3)):
                tt(sh(s1), sh(inv), sh(wsrc), OP.mult)
                tt(vdst_, sh(s1), vdst_, OP.add)

            # --- store (planar, mirror of the load) ---
            osrc = vrt[:].rearrange("p (c m) -> p c m", c=4)
            nc.gpsimd.dma_start(out=planar_ap(outp_d, gb), in_=osrc)

    if split_waits:
        ph.seed_dve_late(nc)
        _retarget_waits(nc, ph.names)
    return nc


def make_bd(x):
    """Block-diag blend stationary (128, 32) fp16 from x (40,)."""
    qm4p1 = np.asarray(x, np.float32).reshape(10, 4)
    qm4p2 = np.zeros_like(qm4p1)
    qm4p2[:, 3] = 1.0
    qm4 = np.concatenate([qm4p1, qm4p2], axis=0)  # (20, 4)
    bd = np.zeros((128, 32), np.float32)
    for nw in range(G_NODES):
        for cc in range(4):
            bd[KW * nw : KW * (nw + 1), 6 * cc + nw] = qm4[:, cc]
    return bd.astype(np.float16)


def pack_weights(w_core, gbases):
    """(npc, 20) fp32 -> (n_grp*3*2048, 128) fp16 xbar-transpose source.

    Window wb of group gb: row c = 512j + 128u + p holds the 20-weight
    rows of nodes gb + 288p + 96wb + 24u + 6j + {0..5} (120 fp16 + 8
    zero pad lanes).
    """
    wh = np.ascontiguousarray(w_core).astype(np.float16)
    n_grp = len(gbases)
    out = np.zeros((n_grp * B_DEF * GPW, 128), np.float16)
    n_full = 0
    while n_full < n_grp and gbases[n_full] == n_full * GRP:
        n_full += 1

    if n_full:
        blk = wh[: n_full * GRP].reshape(n_full, 128, B_DEF, 4, 4, 6, 20)
        # (g, p, wb, u, j, nw, k) -> (g, wb, j, u, p, (nw k))
        out[: n_full * B_DEF * GPW, :120] = blk.transpose(
            0, 2, 4, 3, 1, 5, 6
        ).reshape(n_full * B_DEF * GPW, 120)
    for g in range(n_full, n_grp):
        gb = gbases[g]
        blk = wh[gb : gb + GRP].reshape(128, B_DEF, 4, 4, 6, 20)
        out[g * B_DEF * GPW : (g + 1) * B_DEF * GPW, :120] = blk.transpose(
            1, 3, 2, 0, 4, 5
        ).reshape(B_DEF * GPW, 120)
    return out


_prog_cache = {}


def _get_program(npc, repeats=1):
    key = (npc, repeats)
    if key not in _prog_cache:
        _prog_cache[key] = build_program(npc, repeats)
    return _prog_cache[key]


def make_in_maps(x, weights, VR, npc=NPC, n_cores=N_CORES, repeats=1):
    weights = np.ascontiguousarray(np.asarray(weights, np.float32))
    VR = np.ascontiguousarray(np.asarray(VR, np.float32))
    bd = make_bd(x)
    ident = np.eye(128, dtype=np.float32)
    gbases = _group_bases(npc)
    in_maps = []
    for i in range(n_cores):
        wt = pack_weights(weights[i * npc : (i + 1) * npc], gbases)
        if repeats > 1:
            wt = np.concatenate(
                [wt, np.zeros((repeats - 1, 128), np.float16)], axis=0)
        vr_core = VR[i * npc * 4 : (i + 1) * npc * 4].reshape(npc, 4)
        in_maps.append(
            {
                "wt": wt,
                "vrp": np.ascontiguousarray(
                    vr_core.T.astype(np.float16)).reshape(-1),
                "bd": bd,
                "idf": ident,
            }
        )
    return in_maps


def run(x, weights, VR, npc=NPC, n_cores=N_CORES, trace=False, repeats=1,
        **_ignored):
    nc = _get_program(npc, repeats)
    in_maps = make_in_maps(x, weights, VR, npc, n_cores, repeats)
    res = run_bass_kernel_spmd(nc, in_maps, list(range(n_cores)), trace=trace)
    outs = []
    for i in range(n_cores):
        op = res.results[i]["outp"].reshape(4, npc)
        outs.append(np.ascontiguousarray(op.T).astype(np.float32).reshape(-1))
    return np.concatenate(outs), res


def kernel(x, weights, VR):
    out, _ = run(x, weights, VR)
    return out
